# revision 7
# baseline (speedup 1.0000x reference)
"""GAT layer (nn_CustomGATLayer) on 8 Trainium2 NeuronCores.

Strategy (per sharding hint): shard rows of the NxN attention matrix across
8 cores; each core owns N/8=1024 query nodes and holds Wh of all N key nodes
replicated.  Per core, scores are computed directly in transposed [key j,
query i] layout so the attention @ Wh matmul needs no on-device transposes:

  q[j,i]  = madj[j,i] + s2[j] + s1[i]          (one DVE scalar_tensor_tensor)
  r[j,i]  = leaky_relu(q)                      (ACT Prelu, alpha fused; some
                                                chunks on DVE (q*0.2) max q)
  p[j,i]  = exp(r)                             (ACT Exp -> float32r)
  acc[i,:] += p[:,iblk].T @ [Wh | 1]           (PE, fp32r, 8 psum accumulators)
  out[i,f] = acc[i,f] / acc[i,256]

madj is a host-prepared additive mask in bf16: 0 where the (self-loop added)
adjacency is nonzero, -512 elsewhere, so exp(leaky_relu(t-512)) ~ e^-100 = 0,
matching the reference's hard masking.  Inputs are rolled per-core so every
core runs an identical program (core c's own rows sit first in its local node
order; sums over keys are permutation invariant).
"""
import numpy as np
import ml_dtypes
from contextlib import ExitStack

import concourse.bacc as bacc
import concourse.mybir as mybir
import concourse.tile as tile
from concourse.bass_utils import run_bass_kernel_spmd

F32 = mybir.dt.float32
F32R = mybir.dt.float32r
BF16 = mybir.dt.bfloat16
AF = mybir.ActivationFunctionType
ALU = mybir.AluOpType

N = 8192
F = 256
NCORES = 8
R = N // NCORES          # 1024 query rows per core
CH = N // 128            # 64 key chunks of 128
IB = R // 128            # 8 query blocks of 128
ALPHA = 0.2
BIG = 512.0
KE = F + 2              # Wh chunk width: 256 feats + ones col + even pad
# chunks whose leaky-relu runs on DVE instead of ACT (load balancing)
DVE_LRELU = frozenset(c for c in range(CH) if c % 8 in (2, 5, 7))


def _build():
    nc = bacc.Bacc("TRN2", target_bir_lowering=False, debug=False)
    xT = nc.dram_tensor("xT", [F, N], F32, kind="ExternalInput").ap()
    W = nc.dram_tensor("W", [F, F], F32, kind="ExternalInput").ap()
    WT = nc.dram_tensor("WT", [F, F], F32, kind="ExternalInput").ap()
    a12 = nc.dram_tensor("a12", [F, 2], F32, kind="ExternalInput").ap()
    madjT = nc.dram_tensor("madjT", [N, R], BF16, kind="ExternalInput").ap()
    out = nc.dram_tensor("out", [R, F], F32, kind="ExternalOutput").ap()
    s1d = nc.dram_tensor("s1d", [R], F32).ap()  # bounce for s1 broadcast

    with tile.TileContext(nc) as tc, ExitStack() as ctx:
        persist = ctx.enter_context(tc.tile_pool(name="persist", bufs=1))
        whe = persist.tile([128, CH * KE], F32R, tag="whe")  # [Wh | 1 | 1] chunks
        s1b = persist.tile([128, R], F32, tag="s1b")              # s1 bcast
        s12sb = persist.tile([128, CH * 2], F32, tag="s12sb")     # (s1, s2) per chunk
        w0 = persist.tile([128, F], F32, tag="w0")
        w1 = persist.tile([128, F], F32, tag="w1")
        wr0 = persist.tile([128, F], F32R, tag="wr0")
        wr1 = persist.tile([128, F], F32R, tag="wr1")
        wt0 = persist.tile([128, F], F32, tag="wt0")
        wt1 = persist.tile([128, F], F32, tag="wt1")
        a12t = persist.tile([128, 2, 2], F32, tag="a12t")
        va0 = persist.tile([128, 2], F32, tag="va0")
        va1 = persist.tile([128, 2], F32, tag="va1")
        ones = persist.tile([128, CH], F32, tag="ones")
        s1row = persist.tile([1, R], F32, tag="s1row")

        nc.sync.dma_start(w0[:], W[0:128, :])
        nc.sync.dma_start(w1[:], W[128:256, :])
        nc.sync.dma_start(wt0[:], WT[0:128, :])
        nc.sync.dma_start(wt1[:], WT[128:256, :])
        nc.sync.dma_start(a12t[:, 0, :], a12[0:128, :])
        nc.sync.dma_start(a12t[:, 1, :], a12[128:256, :])
        nc.vector.tensor_copy(wr0[:], w0[:])
        nc.vector.tensor_copy(wr1[:], w1[:])
        nc.vector.memset(ones[:], 1.0)

        # ---- phase 1: Wh = x@W (fp32r), s1/s2 = x@(W@a) (fp32) ----
        with tc.tile_pool(name="xstage", bufs=6) as xpool, \
             tc.tile_pool(name="ph1ps", bufs=1, space="PSUM") as p1ps:
            # va = W @ a  (contraction over output-feature axis, lhsT = W^T)
            wts = (wt0, wt1)
            for kb, va in enumerate((va0, va1)):
                vps = p1ps.tile([128, 2], F32, tag="vps")
                for fc in range(2):
                    nc.tensor.matmul(vps[:], wts[fc][:, kb * 128:(kb + 1) * 128],
                                     a12t[:, fc, :], start=(fc == 0), stop=(fc == 1))
                nc.scalar.copy(va[:], vps[:])

            s12ps = p1ps.tile([128, CH * 2], F32, tag="s12ps")
            for nb in range(CH):
                xt0 = xpool.tile([128, 128], F32, tag="xt0")
                nc.sync.dma_start(xt0[:], xT[0:128, nb * 128:(nb + 1) * 128])
                xt1 = xpool.tile([128, 128], F32, tag="xt1")
                nc.sync.dma_start(xt1[:], xT[128:256, nb * 128:(nb + 1) * 128])
                xr0 = xpool.tile([128, 128], F32R, tag="xr0")
                nc.vector.tensor_copy(xr0[:], xt0[:])
                xr1 = xpool.tile([128, 128], F32R, tag="xr1")
                nc.vector.tensor_copy(xr1[:], xt1[:])

                # s12 in exact fp32
                nc.tensor.matmul(s12ps[:, nb * 2:nb * 2 + 2], xt0[:], va0[:],
                                 start=True, stop=False)
                nc.tensor.matmul(s12ps[:, nb * 2:nb * 2 + 2], xt1[:], va1[:],
                                 start=False, stop=True)
                # Wh in fp32r
                whps = p1ps.tile([128, F], F32, tag="whps", bufs=4)
                nc.tensor.matmul(whps[:], xr0[:], wr0[:], start=True, stop=False)
                nc.tensor.matmul(whps[:], xr1[:], wr1[:], start=False, stop=True)
                dst = whe[:, nb * KE:nb * KE + F]
                if nb % 2 == 0:
                    nc.scalar.copy(dst, whps[:])
                else:
                    nc.vector.tensor_copy(dst, whps[:])

            nc.vector.tensor_copy(s12sb[:], s12ps[:])

        # ones column of [Wh | 1]
        whe3 = whe[:].rearrange("p (c k) -> p c k", k=KE)
        nc.scalar.copy(whe3[:, :, F], ones[:])
        nc.scalar.copy(whe3[:, :, F + 1], ones[:])

        # s1 of own rows (chunks 0..7) -> dram -> partition-broadcast tile
        s12v = s12sb[:].rearrange("p (c t) -> p c t", t=2)
        nc.sync.dma_start(s1d.rearrange("(c p) -> p c", p=128), s12v[:, 0:IB, 0])
        nc.sync.dma_start(s1row[:], s1d.rearrange("(o r) -> o r", o=1))
        nc.gpsimd.partition_broadcast(s1b[:], s1row[:])

        # ---- phase 2: masked exp scores + accumulation matmuls ----
        acc_pool = ctx.enter_context(tc.tile_pool(name="acc", bufs=1, space="PSUM"))
        accs = [acc_pool.tile([128, KE], F32, tag=f"acc{i}", name=f"acc{i}")
                for i in range(IB)]
        mpool = ctx.enter_context(tc.tile_pool(name="madj", bufs=4))
        qpool = ctx.enter_context(tc.tile_pool(name="q", bufs=2))
        rpool = ctx.enter_context(tc.tile_pool(name="r", bufs=2))
        ppool = ctx.enter_context(tc.tile_pool(name="p", bufs=3))

        for c in range(CH):
            madj = mpool.tile([128, R], BF16, tag="madj")
            nc.sync.dma_start(madj[:], madjT[c * 128:(c + 1) * 128, :])
            q = qpool.tile([128, R], F32, tag="q")
            nc.vector.scalar_tensor_tensor(q[:], madj[:], s12v[:, c, 1:2], s1b[:],
                                           op0=ALU.add, op1=ALU.add)
            r = rpool.tile([128, R], F32, tag="r")
            if c in DVE_LRELU:
                nc.vector.scalar_tensor_tensor(r[:], q[:], ALPHA, q[:],
                                               op0=ALU.mult, op1=ALU.max)
            else:
                nc.scalar.activation(r[:], q[:], AF.Prelu, bias=0.0, scale=1.0,
                                     alpha=ALPHA)
            p = ppool.tile([128, R], F32R, tag="p")
            nc.scalar.activation(p[:], r[:], AF.Exp)
            rhs = whe[:, c * KE:(c + 1) * KE]
            for ib in range(IB):
                nc.tensor.matmul(accs[ib][:], p[:, ib * 128:(ib + 1) * 128], rhs,
                                 start=(c == 0), stop=(c == CH - 1))

        # ---- phase 3: normalize + store ----
        opool = ctx.enter_context(tc.tile_pool(name="o", bufs=2))
        rcpool = ctx.enter_context(tc.tile_pool(name="rc", bufs=2))
        for ib in range(IB):
            rec = rcpool.tile([128, 1], F32, tag="rec")
            nc.vector.reciprocal(rec[:], accs[ib][:, F:F + 1])
            ot = opool.tile([128, F], F32, tag="ot")
            nc.vector.tensor_scalar_mul(ot[:], accs[ib][:, 0:F], rec[:])
            nc.sync.dma_start(out[ib * 128:(ib + 1) * 128, :], ot[:])

    nc.compile()
    return nc


_CACHE = {}


def _get_nc():
    if "nc" not in _CACHE:
        _CACHE["nc"] = _build()
    return _CACHE["nc"]


def kernel(adj, x, W, a):
    adj = np.asarray(adj, dtype=np.float32)
    x = np.asarray(x, dtype=np.float32)
    W = np.asarray(W, dtype=np.float32)
    a = np.asarray(a, dtype=np.float32)

    WTc = np.ascontiguousarray(W.T)
    a12 = np.ascontiguousarray(np.stack([a[:F, 0], a[F:, 0]], axis=1))  # [F, 2]
    idx = np.arange(R)

    in_maps = []
    for c in range(NCORES):
        shift = c * R
        xT = np.ascontiguousarray(np.roll(x, -shift, axis=0).T)  # [F, N]
        rows = np.roll(adj[shift:shift + R, :], -shift, axis=1)  # [R, N]
        rows[idx, idx] = 1.0                                     # self loops
        madjT = np.ascontiguousarray(
            np.where(rows > 0, 0.0, -BIG).T.astype(ml_dtypes.bfloat16))
        in_maps.append({"xT": xT, "W": W, "WT": WTc, "a12": a12, "madjT": madjT})

    res = run_bass_kernel_spmd(_get_nc(), in_maps, list(range(NCORES)))
    return np.concatenate([r["out"] for r in res.results], axis=0)


if __name__ == "__main__":
    rng = np.random.default_rng(0)
    adj = (rng.integers(0, 2, (N, N))).astype(np.float32)
    x = rng.normal(size=(N, F)).astype(np.float32)
    W = rng.normal(size=(F, F)).astype(np.float32) * 0.1
    a = rng.normal(size=(2 * F, 1)).astype(np.float32) * 0.1
    out = kernel(adj, x, W, a)
    print(out.shape, out.dtype)
